# revision 6
# baseline (speedup 1.0000x reference)
"""Trainium2 Bass kernel for nn_EncoderSimilarity (block-cosine similarity).

sims[a,b] = sum over block-granularities {128, 256} of
            sum_t max_v ( l2norm(img_block_v) . l2norm(cap_block_t) )

Sharding: img rows (axis a) split 8 ways across cores, cap replicated;
each core computes its [256, 2048] slice of sims.

Device algorithm per core:
  - Block-l2-normalize img slice and cap at granularities 128/256 (the
    reference's global cap l2norm cancels inside the block norm; error ~1e-9).
  - Cast normalized operands to bf16, transpose to [c, b] layout via DMA
    xbar transpose (DRAM round-trip) so the contraction dim is on partitions.
  - Logits via bf16 matmuls; max-over-v uses a relu-diff decomposition
      max(L0, L1) = L1 + relu(L0 - L1),
    where L0-L1 comes directly from a matmul with differenced img weights,
    so ScalarE (relu) shares the PSUM drain work with VectorE (add/max).
  - t-sums accumulate in fp32 via a strided reduce over staged bf16 maxes.
"""
import sys

if "/opt/trn_rl_repo" not in sys.path:
    sys.path.insert(0, "/opt/trn_rl_repo")

from contextlib import ExitStack

import numpy as np

N_CORES = 8
A, B, C = 2048, 2048, 1024
A_PER = A // N_CORES          # 256 img rows per core
NQ = 4                        # b processed in quarters of 512
BQ = B // NQ                  # 512

# engine-assignment tuning knobs
SCALES_ENGINE = "gpsimd"      # normalization scale ops: "vector" | "gpsimd"
L2_GPSIMD_MOD = 0             # gpsimd tensor_tensor unsupported by this walrus
L3_256_GPSIMD = False         # gpsimd tensor_tensor unsupported by this walrus


def _build_kernel():
    import concourse.bass as bass
    import concourse.tile as tile
    from concourse import mybir

    F32 = mybir.dt.float32
    BF16 = mybir.dt.bfloat16
    Alu = mybir.AluOpType
    Act = mybir.ActivationFunctionType
    Ax = mybir.AxisListType

    nc = bass.Bass(
        trn_type="TRN2",
        target_bir_lowering=False,
        debug=False,
        num_devices=N_CORES,
    )
    img_d = nc.dram_tensor("img", [A_PER, C], F32, kind="ExternalInput").ap()
    cap_d = nc.dram_tensor("cap", [B, C], F32, kind="ExternalInput").ap()
    out_d = nc.dram_tensor("sims", [A_PER, B], F32, kind="ExternalOutput").ap()

    with tile.TileContext(nc) as tc, ExitStack() as ctx:
        _body(ctx, tc, out_d, img_d, cap_d, F32, BF16, Alu, Act, Ax)
    return nc


def _body(ctx, tc, out_d, img_d, cap_d, F32, BF16, Alu, Act, Ax):
    import concourse.bass as bass
    nc = tc.nc

    dram = ctx.enter_context(tc.tile_pool(name="dram", bufs=1, space="DRAM"))
    persist = ctx.enter_context(tc.tile_pool(name="persist", bufs=1))
    norm = ctx.enter_context(tc.tile_pool(name="norm", bufs=2))
    small = ctx.enter_context(tc.tile_pool(name="small", bufs=3))
    stage = ctx.enter_context(tc.tile_pool(name="stage", bufs=2))
    drain = ctx.enter_context(tc.tile_pool(name="drain", bufs=3))
    psum = ctx.enter_context(tc.tile_pool(name="psum", bufs=2, space="PSUM"))

    # ---------------- normalization helper (natural [n, c] layout) -------------
    def normalize_tile(x_f32, n128_out, n256_out):
        """x_f32: [128, 1024] fp32 -> block-l2-normalized bf16 tiles (128/256)."""
        sq = norm.tile([128, C], F32, tag="sq")
        nc.scalar.activation(sq[:], x_f32[:], Act.Square)
        s128 = small.tile([128, 8], F32, tag="s128")
        nc.vector.reduce_sum(
            s128[:], sq.rearrange("p (j c) -> p j c", c=128), axis=Ax.X
        )
        s256 = small.tile([128, 4], F32, tag="s256")
        nc.vector.tensor_tensor(
            s256[:],
            s128.rearrange("p (k two) -> p k two", two=2)[:, :, 0],
            s128.rearrange("p (k two) -> p k two", two=2)[:, :, 1],
            op=Alu.add,
        )
        rt128 = small.tile([128, 8], F32, tag="rt128")
        nc.scalar.activation(rt128[:], s128[:], Act.Sqrt)
        inv128 = small.tile([128, 8], F32, tag="inv128")
        nc.vector.reciprocal(inv128[:], rt128[:])
        rt256 = small.tile([128, 4], F32, tag="rt256")
        nc.scalar.activation(rt256[:], s256[:], Act.Sqrt)
        inv256 = small.tile([128, 4], F32, tag="inv256")
        nc.vector.reciprocal(inv256[:], rt256[:])
        seng = nc.gpsimd if SCALES_ENGINE == "gpsimd" else nc.vector
        for j in range(8):
            seng.tensor_scalar_mul(
                n128_out[:, j * 128:(j + 1) * 128],
                x_f32[:, j * 128:(j + 1) * 128],
                inv128[:, j:j + 1],
            )
        for k in range(4):
            seng.tensor_scalar_mul(
                n256_out[:, k * 256:(k + 1) * 256],
                x_f32[:, k * 256:(k + 1) * 256],
                inv256[:, k:k + 1],
            )

    # ---------------- img prep -> transposed bf16 weight tiles -----------------
    # normalized img in natural layout
    img_n128 = persist.tile([128, 2, C], BF16, tag="img_n128")   # [a-tile][a, c]
    img_n256 = persist.tile([128, 2, C], BF16, tag="img_n256")
    img_dn128 = persist.tile([128, 2, 512], BF16, tag="img_dn128")  # 4 pair-diffs
    img_dn256 = persist.tile([128, 2, 512], BF16, tag="img_dn256")  # 2 pair-diffs x 256
    for at in range(2):
        x = norm.tile([128, C], F32, tag="img_in")
        nc.sync.dma_start(x[:], img_d[at * 128:(at + 1) * 128, :])
        normalize_tile(x, img_n128[:, at, :], img_n256[:, at, :])
        # pair diffs on normalized bf16 data (even - odd blocks)
        nc.vector.tensor_tensor(
            img_dn128.rearrange("p t (i c) -> p t i c", c=128)[:, at],
            img_n128.rearrange("p t (v c) -> p t v c", c=128)[:, at, 0::2, :],
            img_n128.rearrange("p t (v c) -> p t v c", c=128)[:, at, 1::2, :],
            op=Alu.subtract,
        )
        nc.vector.tensor_tensor(
            img_dn256.rearrange("p t (i c) -> p t i c", c=256)[:, at],
            img_n256.rearrange("p t (v c) -> p t v c", c=256)[:, at, 0::2, :],
            img_n256.rearrange("p t (v c) -> p t v c", c=256)[:, at, 1::2, :],
            op=Alu.subtract,
        )

    # stage img to DRAM and transpose back to [c, a] layout
    scr_i128 = dram.tile([A_PER, C], BF16, tag="scr_i128")
    scr_i256 = dram.tile([A_PER, C], BF16, tag="scr_i256")
    scr_d128 = dram.tile([A_PER, 512], BF16, tag="scr_d128")
    scr_d256 = dram.tile([A_PER, 512], BF16, tag="scr_d256")
    for at in range(2):
        sl = slice(at * 128, (at + 1) * 128)
        nc.sync.dma_start(scr_i128[sl, :], img_n128[:, at, :])
        nc.sync.dma_start(scr_i256[sl, :], img_n256[:, at, :])
        nc.sync.dma_start(scr_d128[sl, :], img_dn128[:, at, :])
        nc.sync.dma_start(scr_d256[sl, :], img_dn256[:, at, :])

    # weight tiles, [c, a] layout: index i = pair 0..3
    wL128 = persist.tile([128, 4, A_PER], BF16, tag="wL128")  # odd chunk 2i+1
    wD128 = persist.tile([128, 4, A_PER], BF16, tag="wD128")
    wL256 = persist.tile([128, 4, A_PER], BF16, tag="wL256")  # [2i+h]: odd v'=2i+1, half h
    wD256 = persist.tile([128, 4, A_PER], BF16, tag="wD256")
    for i in range(4):
        j = 2 * i + 1  # odd 128-chunk
        nc.sync.dma_start_transpose(wL128[:, i, :], scr_i128[:, j * 128:(j + 1) * 128])
        nc.sync.dma_start_transpose(wD128[:, i, :], scr_d128[:, i * 128:(i + 1) * 128])
    for i in range(2):       # pair of 256-blocks: odd v' = 2i+1
        for h in range(2):   # 128-half of the 256-block
            j = (2 * i + 1) * 2 + h
            nc.sync.dma_start_transpose(
                wL256[:, 2 * i + h, :], scr_i256[:, j * 128:(j + 1) * 128]
            )
            nc.sync.dma_start_transpose(
                wD256[:, 2 * i + h, :], scr_d256[:, (2 * i) * 128 + h * 128:(2 * i) * 128 + (h + 1) * 128]
            )

    # ---------------- cap prep (per quarter) + main loop, interleaved ----------
    scr_c128 = dram.tile([B, C], BF16, tag="scr_c128")
    scr_c256 = dram.tile([B, C], BF16, tag="scr_c256")
    for q in range(NQ):
        c128q = persist.tile([128, 8, BQ], BF16, tag=f"capT128_{q}")
        c256q = persist.tile([128, 8, BQ], BF16, tag=f"capT256_{q}")
        for r in range(4):  # row-tiles within quarter
            row0 = q * BQ + r * 128
            x = norm.tile([128, C], F32, tag="cap_in")
            nc.sync.dma_start(x[:], cap_d[row0:row0 + 128, :])
            n128 = norm.tile([128, C], BF16, tag="cap_n128")
            n256 = norm.tile([128, C], BF16, tag="cap_n256")
            normalize_tile(x, n128, n256)
            nc.sync.dma_start(scr_c128[row0:row0 + 128, :], n128[:])
            nc.sync.dma_start(scr_c256[row0:row0 + 128, :], n256[:])
        for j in range(8):
            nc.sync.dma_start_transpose(
                c128q[:, j, :], scr_c128[q * BQ:(q + 1) * BQ, j * 128:(j + 1) * 128]
            )
            nc.sync.dma_start_transpose(
                c256q[:, j, :], scr_c256[q * BQ:(q + 1) * BQ, j * 128:(j + 1) * 128]
            )

        for at in range(2):
            asl = slice(at * 128, (at + 1) * 128)
            m_stage = stage.tile([128, 12, BQ], BF16, tag="m_stage")
            # ---- 128-blocks: t = cap chunk, v pairs (2g+i) ----
            for t in range(8):
                m_g = []
                for g in range(2):
                    pL = psum.tile([128, 2, BQ], F32, tag="pL")
                    pD = psum.tile([128, 2, BQ], F32, tag="pD")
                    for i in range(2):
                        pair = 2 * g + i
                        rhs = c128q[:, t, :]
                        nc.tensor.matmul(pL[:, i, :], wL128[:, pair, asl], rhs,
                                         start=True, stop=True)
                        nc.tensor.matmul(pD[:, i, :], wD128[:, pair, asl], rhs,
                                         start=True, stop=True)
                    r = drain.tile([128, 2, BQ], BF16, tag="r")
                    nc.scalar.activation(r[:], pD[:], Act.Relu)
                    m = drain.tile([128, 2, BQ], BF16, tag="m", bufs=4)
                    nc.vector.tensor_tensor(m[:], r[:], pL[:], op=Alu.add)
                    m_g.append(m)
                mm = drain.tile([128, 2, BQ], BF16, tag="mm", bufs=2)
                l2eng = (
                    nc.gpsimd
                    if (L2_GPSIMD_MOD and t % L2_GPSIMD_MOD == 0)
                    else nc.vector
                )
                l2eng.tensor_tensor(mm[:], m_g[0][:], m_g[1][:], op=Alu.max)
                nc.vector.tensor_tensor(
                    m_stage[:, t, :], mm[:, 0, :], mm[:, 1, :], op=Alu.max
                )
            # ---- 256-blocks: t' = cap 256-chunk, v' pairs ----
            for tp in range(4):
                pL = psum.tile([128, 2, BQ], F32, tag="pL")
                pD = psum.tile([128, 2, BQ], F32, tag="pD")
                for i in range(2):
                    for h in range(2):
                        rhs = c256q[:, 2 * tp + h, :]
                        nc.tensor.matmul(pL[:, i, :], wL256[:, 2 * i + h, asl], rhs,
                                         start=(h == 0), stop=(h == 1))
                        nc.tensor.matmul(pD[:, i, :], wD256[:, 2 * i + h, asl], rhs,
                                         start=(h == 0), stop=(h == 1))
                r = drain.tile([128, 2, BQ], BF16, tag="r")
                nc.scalar.activation(r[:], pD[:], Act.Relu)
                m = drain.tile([128, 2, BQ], BF16, tag="m", bufs=4)
                nc.vector.tensor_tensor(m[:], r[:], pL[:], op=Alu.add)
                l3eng = nc.gpsimd if L3_256_GPSIMD else nc.vector
                l3eng.tensor_tensor(
                    m_stage[:, 8 + tp, :], m[:, 0, :], m[:, 1, :], op=Alu.max
                )
            # ---- t-sum via contiguous slab add-tree and writeback ----
            # 12 -> 6 (bf16) -> 3 (fp32) -> 1
            s1 = drain.tile([128, 6, BQ], BF16, tag="s1", bufs=2)
            nc.vector.tensor_tensor(
                s1[:], m_stage[:, 0:6, :], m_stage[:, 6:12, :], op=Alu.add
            )
            s2 = drain.tile([128, 3, BQ], F32, tag="s2", bufs=2)
            nc.vector.tensor_tensor(
                s2[:], s1[:, 0:3, :], s1[:, 3:6, :], op=Alu.add
            )
            acc = drain.tile([128, BQ], F32, tag="acc", bufs=2)
            nc.vector.tensor_tensor(
                acc[:], s2[:, 0, :], s2[:, 1, :], op=Alu.add
            )
            nc.vector.tensor_tensor(
                acc[:], acc[:], s2[:, 2, :], op=Alu.add
            )
            nc.sync.dma_start(out_d[asl, q * BQ:(q + 1) * BQ], acc[:])


_NC_CACHE = None




# ---------------------------------------------------------------------------
# Workaround: this container's walrus build rejects instructions with more
# than one sync-wait condition ("Too many sync wait commands").  Split the
# extra waits onto sequencer-only RegisterMove carrier instructions in a BIR
# post-pass, and monkeypatch the compile entry points to apply it.
import json as _json


def _split_multiwaits(bir_bytes: bytes) -> bytes:
    m = _json.loads(bir_bytes)
    uid = [0]

    def carrier(engine, wait, debug):
        uid[0] += 1
        return {
            "debug": debug,
            "engine": engine,
            "ins": [{"dtype": "int32", "kind": "imm_value", "value": 0}],
            "outs": [{"dtype": "int32", "kind": "register_access",
                      "regref": f"{engine}_zero"}],
            "name": f"I-wsplit-{uid[0]}",
            "opcode": "RegisterMove",
            "sync_info": {"on_update": [], "on_wait": [wait]},
        }

    for f in m["functions"]:
        for bb in f["blocks"]:
            out = []
            for inst in bb["instructions"]:
                si = inst.get("sync_info")
                waits = (si or {}).get("on_wait") or []
                eng = inst.get("engine")
                if len(waits) > 1 and eng and eng != "Unassigned":
                    for w in waits[:-1]:
                        out.append(carrier(eng, w, inst.get("debug", 0)))
                    si["on_wait"] = [waits[-1]]
                out.append(inst)
            bb["instructions"] = out
    return _json.dumps(m).encode()


def _install_birpatch():
    import concourse.bass_utils as bu
    import concourse.bass2jax as b2j

    if getattr(bu.compile_bir_kernel, "_wsplit_wrapped", False):
        return
    orig = bu.compile_bir_kernel

    def wrapped(bir_json: bytes, tmpdir: str, neff_name="file.neff"):
        return orig(_split_multiwaits(bir_json), tmpdir, neff_name=neff_name)

    wrapped._wsplit_wrapped = True
    bu.compile_bir_kernel = wrapped
    b2j.compile_bir_kernel = wrapped


def kernel(img_emb: np.ndarray, cap_emb: np.ndarray) -> np.ndarray:
    _install_birpatch()
    from concourse.bass_utils import run_bass_kernel_spmd

    global _NC_CACHE
    if _NC_CACHE is None:
        _NC_CACHE = _build_kernel()
    nc = _NC_CACHE

    img = np.ascontiguousarray(np.asarray(img_emb, dtype=np.float32))
    cap = np.ascontiguousarray(np.asarray(cap_emb, dtype=np.float32))
    in_maps = [
        {"img": img[k * A_PER:(k + 1) * A_PER], "cap": cap} for k in range(N_CORES)
    ]
    res = run_bass_kernel_spmd(nc, in_maps, core_ids=list(range(N_CORES)))
    return np.concatenate([r["sims"] for r in res.results], axis=0)


if __name__ == "__main__":
    rng = np.random.default_rng(0)
    img = rng.normal(size=(A, C)).astype(np.float32)
    cap = rng.normal(size=(B, C)).astype(np.float32)
    out = kernel(img, cap)
    print("out", out.shape, out.dtype, float(out.min()), float(out.max()))


# revision 7
# speedup vs baseline: 1.9712x; 1.9712x over previous
"""Trainium2 Bass kernel for nn_EncoderSimilarity (block-cosine similarity).

sims[a,b] = sum over block-granularities {128, 256} of
            sum_t max_v ( l2norm(img_block_v) . l2norm(cap_block_t) )

Sharding: img rows (axis a) split 8 ways across cores, cap replicated;
each core computes its [256, 2048] slice of sims.

Device algorithm per core:
  - Block-l2-normalize img slice and cap at granularities 128/256 (the
    reference's global cap l2norm cancels inside the block norm; error ~1e-9).
  - Cast normalized operands to bf16, transpose to [c, b] layout via DMA
    xbar transpose (DRAM round-trip) so the contraction dim is on partitions.
  - Logits via bf16 matmuls; max-over-v uses a relu-diff decomposition
      max(L0, L1) = L1 + relu(L0 - L1),
    where L0-L1 comes directly from a matmul with differenced img weights,
    so ScalarE (relu) shares the PSUM drain work with VectorE (add/max).
  - t-sums accumulate in fp32 via a strided reduce over staged bf16 maxes.
"""
import sys

if "/opt/trn_rl_repo" not in sys.path:
    sys.path.insert(0, "/opt/trn_rl_repo")

from contextlib import ExitStack

import numpy as np

N_CORES = 8
A, B, C = 2048, 2048, 1024
A_PER = A // N_CORES          # 256 img rows per core
NQ = 4                        # b processed in quarters of 512
BQ = B // NQ                  # 512

# engine-assignment tuning knobs
SCALES_ENGINE = "gpsimd"      # normalization scale ops: "vector" | "gpsimd"
L2_GPSIMD_MOD = 0             # gpsimd tensor_tensor unsupported by this walrus
L3_256_GPSIMD = False         # gpsimd tensor_tensor unsupported by this walrus


def _build_kernel():
    import concourse.bass as bass
    import concourse.tile as tile
    from concourse import mybir

    F32 = mybir.dt.float32
    BF16 = mybir.dt.bfloat16
    Alu = mybir.AluOpType
    Act = mybir.ActivationFunctionType
    Ax = mybir.AxisListType

    nc = bass.Bass(
        trn_type="TRN2",
        target_bir_lowering=False,
        debug=False,
        num_devices=N_CORES,
    )
    img_d = nc.dram_tensor("img", [A_PER, C], F32, kind="ExternalInput").ap()
    cap_d = nc.dram_tensor("cap", [B, C], F32, kind="ExternalInput").ap()
    ident_d = nc.dram_tensor("ident", [128, 128], BF16, kind="ExternalInput").ap()
    out_d = nc.dram_tensor("sims", [A_PER, B], F32, kind="ExternalOutput").ap()

    with tile.TileContext(nc) as tc, ExitStack() as ctx:
        _body(ctx, tc, out_d, img_d, cap_d, ident_d, F32, BF16, Alu, Act, Ax)
    return nc


def _body(ctx, tc, out_d, img_d, cap_d, ident_d, F32, BF16, Alu, Act, Ax):
    import concourse.bass as bass
    nc = tc.nc

    dram = ctx.enter_context(tc.tile_pool(name="dram", bufs=1, space="DRAM"))
    persist = ctx.enter_context(tc.tile_pool(name="persist", bufs=1))
    norm = ctx.enter_context(tc.tile_pool(name="norm", bufs=2))
    small = ctx.enter_context(tc.tile_pool(name="small", bufs=3))
    stage = ctx.enter_context(tc.tile_pool(name="stage", bufs=2))
    drain = ctx.enter_context(tc.tile_pool(name="drain", bufs=3))
    psum = ctx.enter_context(tc.tile_pool(name="psum", bufs=2, space="PSUM"))

    # ---------------- normalization helper (natural [n, c] layout) -------------
    def normalize_tile(x_f32, n128_out, n256_out):
        """x_f32: [128, 1024] fp32 -> block-l2-normalized bf16 tiles (128/256)."""
        sq = norm.tile([128, C], F32, tag="sq")
        nc.scalar.activation(sq[:], x_f32[:], Act.Square)
        s128 = small.tile([128, 8], F32, tag="s128")
        nc.vector.reduce_sum(
            s128[:], sq.rearrange("p (j c) -> p j c", c=128), axis=Ax.X
        )
        s256 = small.tile([128, 4], F32, tag="s256")
        nc.vector.tensor_tensor(
            s256[:],
            s128.rearrange("p (k two) -> p k two", two=2)[:, :, 0],
            s128.rearrange("p (k two) -> p k two", two=2)[:, :, 1],
            op=Alu.add,
        )
        rt128 = small.tile([128, 8], F32, tag="rt128")
        nc.scalar.activation(rt128[:], s128[:], Act.Sqrt)
        inv128 = small.tile([128, 8], F32, tag="inv128")
        nc.vector.reciprocal(inv128[:], rt128[:])
        rt256 = small.tile([128, 4], F32, tag="rt256")
        nc.scalar.activation(rt256[:], s256[:], Act.Sqrt)
        inv256 = small.tile([128, 4], F32, tag="inv256")
        nc.vector.reciprocal(inv256[:], rt256[:])
        for j in range(8):
            nc.scalar.mul(
                n128_out[:, j * 128:(j + 1) * 128],
                x_f32[:, j * 128:(j + 1) * 128],
                inv128[:, j:j + 1],
            )
        for k in range(4):
            nc.vector.tensor_scalar_mul(
                n256_out[:, k * 256:(k + 1) * 256],
                x_f32[:, k * 256:(k + 1) * 256],
                inv256[:, k:k + 1],
            )

    # ---------------- img prep -> transposed bf16 weight tiles -----------------
    # normalized img in natural layout
    img_n128 = persist.tile([128, 2, C], BF16, tag="img_n128")   # [a-tile][a, c]
    img_n256 = persist.tile([128, 2, C], BF16, tag="img_n256")
    img_dn128 = persist.tile([128, 2, 512], BF16, tag="img_dn128")  # 4 pair-diffs
    img_dn256 = persist.tile([128, 2, 512], BF16, tag="img_dn256")  # 2 pair-diffs x 256
    for at in range(2):
        x = norm.tile([128, C], F32, tag="img_in")
        nc.sync.dma_start(x[:], img_d[at * 128:(at + 1) * 128, :])
        normalize_tile(x, img_n128[:, at, :], img_n256[:, at, :])
        # pair diffs on normalized bf16 data (even - odd blocks)
        nc.vector.tensor_tensor(
            img_dn128.rearrange("p t (i c) -> p t i c", c=128)[:, at],
            img_n128.rearrange("p t (v c) -> p t v c", c=128)[:, at, 0::2, :],
            img_n128.rearrange("p t (v c) -> p t v c", c=128)[:, at, 1::2, :],
            op=Alu.subtract,
        )
        nc.vector.tensor_tensor(
            img_dn256.rearrange("p t (i c) -> p t i c", c=256)[:, at],
            img_n256.rearrange("p t (v c) -> p t v c", c=256)[:, at, 0::2, :],
            img_n256.rearrange("p t (v c) -> p t v c", c=256)[:, at, 1::2, :],
            op=Alu.subtract,
        )

    # identity for PE transposes
    ident = persist.tile([128, 128], BF16, tag="ident")
    nc.sync.dma_start(ident[:], ident_d[:])

    # weight tiles, [c, a] layout, via PE transposes (PE is idle in the prologue)
    wL128 = persist.tile([128, 4, A_PER], BF16, tag="wL128")  # odd chunk 2i+1
    wD128 = persist.tile([128, 4, A_PER], BF16, tag="wD128")
    wL256 = persist.tile([128, 4, A_PER], BF16, tag="wL256")  # [2i+h]: odd v'=2i+1, half h
    wD256 = persist.tile([128, 4, A_PER], BF16, tag="wD256")
    for at in range(2):
        asl = slice(at * 128, (at + 1) * 128)
        groups = [
            (wL128, [img_n128[:, at, (2 * i + 1) * 128:(2 * i + 2) * 128] for i in range(4)]),
            (wD128, [img_dn128[:, at, i * 128:(i + 1) * 128] for i in range(4)]),
            (wL256, [img_n256[:, at, ((2 * i + 1) * 2 + h) * 128:((2 * i + 1) * 2 + h + 1) * 128]
                     for i in range(2) for h in range(2)]),
            (wD256, [img_dn256[:, at, k * 128:(k + 1) * 128] for k in range(4)]),
        ]
        for gi, (dstT, srcs) in enumerate(groups):
            pt = psum.tile([128, 4, 128], BF16, tag="pL")
            for k, s in enumerate(srcs):
                nc.tensor.transpose(pt[:, k, :], s, ident[:])
            eng = nc.vector if gi % 2 == 0 else nc.scalar
            if gi % 2 == 0:
                nc.vector.tensor_copy(dstT[:, 0:4, asl], pt[:])
            else:
                nc.scalar.copy(dstT[:, 0:4, asl], pt[:])

    # ---------------- cap prep (per quarter) + main loop, interleaved ----------
    scr_c128 = dram.tile([B, C], BF16, tag="scr_c128")
    scr_c256 = dram.tile([B, C], BF16, tag="scr_c256")
    for q in range(NQ):
        c128q = persist.tile([128, 8, BQ], BF16, tag=f"capT128_{q}")
        c256q = persist.tile([128, 8, BQ], BF16, tag=f"capT256_{q}")
        for r in range(4):  # row-tiles within quarter
            row0 = q * BQ + r * 128
            x = norm.tile([128, C], F32, tag="cap_in")
            nc.sync.dma_start(x[:], cap_d[row0:row0 + 128, :])
            n128 = norm.tile([128, C], BF16, tag="cap_n128")
            n256 = norm.tile([128, C], BF16, tag="cap_n256")
            normalize_tile(x, n128, n256)
            if q == 0:
                # PE transposes straight from SBUF: no DRAM roundtrip latency
                for half, (srcT, dstq) in enumerate(((n128, c128q), (n256, c256q))):
                    for jg in range(2):
                        pt = psum.tile([128, 4, 128], BF16, tag="pL")
                        for k in range(4):
                            j = jg * 4 + k
                            nc.tensor.transpose(
                                pt[:, k, :], srcT[:, j * 128:(j + 1) * 128], ident[:]
                            )
                        dst = dstq[:, jg * 4:(jg + 1) * 4, r * 128:(r + 1) * 128]
                        if (half + jg) % 2 == 0:
                            nc.vector.tensor_copy(dst, pt[:])
                        else:
                            nc.scalar.copy(dst, pt[:])
            else:
                nc.sync.dma_start(scr_c128[row0:row0 + 128, :], n128[:])
                nc.sync.dma_start(scr_c256[row0:row0 + 128, :], n256[:])
        if q > 0:
            for j in range(8):
                nc.sync.dma_start_transpose(
                    c128q[:, j, :], scr_c128[q * BQ:(q + 1) * BQ, j * 128:(j + 1) * 128]
                )
                nc.sync.dma_start_transpose(
                    c256q[:, j, :], scr_c256[q * BQ:(q + 1) * BQ, j * 128:(j + 1) * 128]
                )

        for at in range(2):
            asl = slice(at * 128, (at + 1) * 128)
            m_stage = stage.tile([128, 12, BQ], BF16, tag="m_stage")
            # ---- 128-blocks: t = cap chunk, v pairs (2g+i) ----
            for t in range(8):
                m_g = []
                for g in range(2):
                    pL = psum.tile([128, 2, BQ], F32, tag="pL")
                    pD = psum.tile([128, 2, BQ], F32, tag="pD")
                    for i in range(2):
                        pair = 2 * g + i
                        rhs = c128q[:, t, :]
                        nc.tensor.matmul(pL[:, i, :], wL128[:, pair, asl], rhs,
                                         start=True, stop=True)
                        nc.tensor.matmul(pD[:, i, :], wD128[:, pair, asl], rhs,
                                         start=True, stop=True)
                    r = drain.tile([128, 2, BQ], BF16, tag="r")
                    nc.scalar.activation(r[:], pD[:], Act.Relu)
                    m = drain.tile([128, 2, BQ], BF16, tag="m", bufs=4)
                    nc.vector.tensor_tensor(m[:], r[:], pL[:], op=Alu.add)
                    m_g.append(m)
                mm = drain.tile([128, 2, BQ], BF16, tag="mm", bufs=2)
                l2eng = (
                    nc.gpsimd
                    if (L2_GPSIMD_MOD and t % L2_GPSIMD_MOD == 0)
                    else nc.vector
                )
                l2eng.tensor_tensor(mm[:], m_g[0][:], m_g[1][:], op=Alu.max)
                nc.vector.tensor_tensor(
                    m_stage[:, t, :], mm[:, 0, :], mm[:, 1, :], op=Alu.max
                )
            # ---- 256-blocks: t' = cap 256-chunk, v' pairs ----
            for tp in range(4):
                pL = psum.tile([128, 2, BQ], F32, tag="pL")
                pD = psum.tile([128, 2, BQ], F32, tag="pD")
                for i in range(2):
                    for h in range(2):
                        rhs = c256q[:, 2 * tp + h, :]
                        nc.tensor.matmul(pL[:, i, :], wL256[:, 2 * i + h, asl], rhs,
                                         start=(h == 0), stop=(h == 1))
                        nc.tensor.matmul(pD[:, i, :], wD256[:, 2 * i + h, asl], rhs,
                                         start=(h == 0), stop=(h == 1))
                r = drain.tile([128, 2, BQ], BF16, tag="r")
                nc.scalar.activation(r[:], pD[:], Act.Relu)
                m = drain.tile([128, 2, BQ], BF16, tag="m", bufs=4)
                nc.vector.tensor_tensor(m[:], r[:], pL[:], op=Alu.add)
                l3eng = nc.gpsimd if L3_256_GPSIMD else nc.vector
                l3eng.tensor_tensor(
                    m_stage[:, 8 + tp, :], m[:, 0, :], m[:, 1, :], op=Alu.max
                )
            # ---- t-sum via contiguous slab add-tree and writeback ----
            # 12 -> 6 (bf16) -> 3 (fp32) -> 1
            s1 = drain.tile([128, 6, BQ], BF16, tag="s1", bufs=2)
            nc.vector.tensor_tensor(
                s1[:], m_stage[:, 0:6, :], m_stage[:, 6:12, :], op=Alu.add
            )
            s2 = drain.tile([128, 3, BQ], F32, tag="s2", bufs=2)
            nc.vector.tensor_tensor(
                s2[:], s1[:, 0:3, :], s1[:, 3:6, :], op=Alu.add
            )
            acc = drain.tile([128, BQ], F32, tag="acc", bufs=2)
            nc.vector.tensor_tensor(
                acc[:], s2[:, 0, :], s2[:, 1, :], op=Alu.add
            )
            nc.vector.tensor_tensor(
                acc[:], acc[:], s2[:, 2, :], op=Alu.add
            )
            nc.sync.dma_start(out_d[asl, q * BQ:(q + 1) * BQ], acc[:])


_NC_CACHE = None




# ---------------------------------------------------------------------------
# Workaround: this container's walrus build rejects instructions with more
# than one sync-wait condition ("Too many sync wait commands").  Split the
# extra waits onto sequencer-only RegisterMove carrier instructions in a BIR
# post-pass, and monkeypatch the compile entry points to apply it.
import json as _json


def _split_multiwaits(bir_bytes: bytes) -> bytes:
    m = _json.loads(bir_bytes)
    uid = [0]

    def carrier(engine, wait, debug):
        uid[0] += 1
        return {
            "debug": debug,
            "engine": engine,
            "ins": [{"dtype": "int32", "kind": "imm_value", "value": 0}],
            "outs": [{"dtype": "int32", "kind": "register_access",
                      "regref": f"{engine}_zero"}],
            "name": f"I-wsplit-{uid[0]}",
            "opcode": "RegisterMove",
            "sync_info": {"on_update": [], "on_wait": [wait]},
        }

    for f in m["functions"]:
        for bb in f["blocks"]:
            out = []
            for inst in bb["instructions"]:
                si = inst.get("sync_info")
                waits = (si or {}).get("on_wait") or []
                eng = inst.get("engine")
                if len(waits) > 1 and eng and eng != "Unassigned":
                    for w in waits[:-1]:
                        out.append(carrier(eng, w, inst.get("debug", 0)))
                    si["on_wait"] = [waits[-1]]
                out.append(inst)
            bb["instructions"] = out
    return _json.dumps(m).encode()


def _install_birpatch():
    import concourse.bass_utils as bu
    import concourse.bass2jax as b2j

    if getattr(bu.compile_bir_kernel, "_wsplit_wrapped", False):
        return
    orig = bu.compile_bir_kernel

    def wrapped(bir_json: bytes, tmpdir: str, neff_name="file.neff"):
        return orig(_split_multiwaits(bir_json), tmpdir, neff_name=neff_name)

    wrapped._wsplit_wrapped = True
    bu.compile_bir_kernel = wrapped
    b2j.compile_bir_kernel = wrapped


def kernel(img_emb: np.ndarray, cap_emb: np.ndarray) -> np.ndarray:
    _install_birpatch()
    from concourse.bass_utils import run_bass_kernel_spmd

    global _NC_CACHE
    if _NC_CACHE is None:
        _NC_CACHE = _build_kernel()
    nc = _NC_CACHE

    import ml_dtypes

    img = np.ascontiguousarray(np.asarray(img_emb, dtype=np.float32))
    cap = np.ascontiguousarray(np.asarray(cap_emb, dtype=np.float32))
    ident = np.eye(128, dtype=ml_dtypes.bfloat16)
    in_maps = [
        {"img": img[k * A_PER:(k + 1) * A_PER], "cap": cap, "ident": ident}
        for k in range(N_CORES)
    ]
    res = run_bass_kernel_spmd(nc, in_maps, core_ids=list(range(N_CORES)))
    return np.concatenate([r["sims"] for r in res.results], axis=0)


if __name__ == "__main__":
    rng = np.random.default_rng(0)
    img = rng.normal(size=(A, C)).astype(np.float32)
    cap = rng.normal(size=(B, C)).astype(np.float32)
    out = kernel(img, cap)
    print("out", out.shape, out.dtype, float(out.min()), float(out.max()))


# revision 8
# speedup vs baseline: 2.3378x; 1.1860x over previous
"""Trainium2 Bass kernel for nn_EncoderSimilarity (block-cosine similarity).

sims[a,b] = sum over block-granularities {128, 256} of
            sum_t max_v ( l2norm(img_block_v) . l2norm(cap_block_t) )

Sharding: img rows (axis a) split 8 ways across cores, cap replicated;
each core computes its [256, 2048] slice of sims.

Device algorithm per core:
  - Block-l2-normalize img slice and cap at granularities 128/256 (the
    reference's global cap l2norm cancels inside the block norm; error ~1e-9).
  - Cast normalized operands to bf16, transpose to [c, b] layout via DMA
    xbar transpose (DRAM round-trip) so the contraction dim is on partitions.
  - Logits via bf16 matmuls; max-over-v uses a relu-diff decomposition
      max(L0, L1) = L1 + relu(L0 - L1),
    where L0-L1 comes directly from a matmul with differenced img weights,
    so ScalarE (relu) shares the PSUM drain work with VectorE (add/max).
  - t-sums accumulate in fp32 via a strided reduce over staged bf16 maxes.
"""
import sys

if "/opt/trn_rl_repo" not in sys.path:
    sys.path.insert(0, "/opt/trn_rl_repo")

from contextlib import ExitStack

import numpy as np

N_CORES = 8
A, B, C = 2048, 2048, 1024
A_PER = A // N_CORES          # 256 img rows per core
NQ = 4                        # b processed in quarters of 512
BQ = B // NQ                  # 512

# engine-assignment tuning knobs
SCALES_ENGINE = "gpsimd"      # normalization scale ops: "vector" | "gpsimd"
L2_GPSIMD_MOD = 0             # gpsimd tensor_tensor unsupported by this walrus
L3_256_GPSIMD = False         # gpsimd tensor_tensor unsupported by this walrus


def _build_kernel():
    import concourse.bass as bass
    import concourse.tile as tile
    from concourse import mybir

    F32 = mybir.dt.float32
    BF16 = mybir.dt.bfloat16
    Alu = mybir.AluOpType
    Act = mybir.ActivationFunctionType
    Ax = mybir.AxisListType

    nc = bass.Bass(
        trn_type="TRN2",
        target_bir_lowering=False,
        debug=False,
        num_devices=N_CORES,
    )
    img_d = nc.dram_tensor("img", [A_PER, C], F32, kind="ExternalInput").ap()
    cap_d = nc.dram_tensor("cap", [B, C], F32, kind="ExternalInput").ap()
    ident_d = nc.dram_tensor("ident", [128, 128], BF16, kind="ExternalInput").ap()
    out_d = nc.dram_tensor("sims", [A_PER, B], F32, kind="ExternalOutput").ap()

    with tile.TileContext(nc) as tc, ExitStack() as ctx:
        _body(ctx, tc, out_d, img_d, cap_d, ident_d, F32, BF16, Alu, Act, Ax)
    return nc


def _body(ctx, tc, out_d, img_d, cap_d, ident_d, F32, BF16, Alu, Act, Ax):
    import concourse.bass as bass
    nc = tc.nc

    dram = ctx.enter_context(tc.tile_pool(name="dram", bufs=1, space="DRAM"))
    persist = ctx.enter_context(tc.tile_pool(name="persist", bufs=1))
    norm = ctx.enter_context(tc.tile_pool(name="norm", bufs=2))
    small = ctx.enter_context(tc.tile_pool(name="small", bufs=3))
    stage = ctx.enter_context(tc.tile_pool(name="stage", bufs=2))
    drain = ctx.enter_context(tc.tile_pool(name="drain", bufs=3))
    psum = ctx.enter_context(tc.tile_pool(name="psum", bufs=2, space="PSUM"))

    # ---------------- normalization helper (natural [n, c] layout) -------------
    def normalize_tile(x_f32, n128_out, n256_out):
        """x_f32: [128, 1024] fp32 -> block-l2-normalized bf16 tiles (128/256)."""
        sq = norm.tile([128, C], F32, tag="sq")
        nc.scalar.activation(sq[:], x_f32[:], Act.Square)
        s128 = small.tile([128, 8], F32, tag="s128")
        nc.vector.reduce_sum(
            s128[:], sq.rearrange("p (j c) -> p j c", c=128), axis=Ax.X
        )
        s256 = small.tile([128, 4], F32, tag="s256")
        nc.vector.tensor_tensor(
            s256[:],
            s128.rearrange("p (k two) -> p k two", two=2)[:, :, 0],
            s128.rearrange("p (k two) -> p k two", two=2)[:, :, 1],
            op=Alu.add,
        )
        rt128 = small.tile([128, 8], F32, tag="rt128")
        nc.scalar.activation(rt128[:], s128[:], Act.Sqrt)
        inv128 = small.tile([128, 8], F32, tag="inv128")
        nc.vector.reciprocal(inv128[:], rt128[:])
        rt256 = small.tile([128, 4], F32, tag="rt256")
        nc.scalar.activation(rt256[:], s256[:], Act.Sqrt)
        inv256 = small.tile([128, 4], F32, tag="inv256")
        nc.vector.reciprocal(inv256[:], rt256[:])
        for j in range(8):
            nc.scalar.mul(
                n128_out[:, j * 128:(j + 1) * 128],
                x_f32[:, j * 128:(j + 1) * 128],
                inv128[:, j:j + 1],
            )
        for k in range(4):
            nc.vector.tensor_scalar_mul(
                n256_out[:, k * 256:(k + 1) * 256],
                x_f32[:, k * 256:(k + 1) * 256],
                inv256[:, k:k + 1],
            )

    # ---------------- img prep -> transposed bf16 weight tiles -----------------
    # normalized img in natural layout
    img_n128 = persist.tile([128, 2, C], BF16, tag="img_n128")   # [a-tile][a, c]
    img_n256 = persist.tile([128, 2, C], BF16, tag="img_n256")
    img_dn128 = persist.tile([128, 2, 512], BF16, tag="img_dn128")  # 4 pair-diffs
    img_dn256 = persist.tile([128, 2, 512], BF16, tag="img_dn256")  # 2 pair-diffs x 256
    for at in range(2):
        x = norm.tile([128, C], F32, tag="img_in")
        nc.sync.dma_start(x[:], img_d[at * 128:(at + 1) * 128, :])
        normalize_tile(x, img_n128[:, at, :], img_n256[:, at, :])
        # pair diffs on normalized bf16 data (even - odd blocks)
        nc.vector.tensor_tensor(
            img_dn128.rearrange("p t (i c) -> p t i c", c=128)[:, at],
            img_n128.rearrange("p t (v c) -> p t v c", c=128)[:, at, 0::2, :],
            img_n128.rearrange("p t (v c) -> p t v c", c=128)[:, at, 1::2, :],
            op=Alu.subtract,
        )
        nc.vector.tensor_tensor(
            img_dn256.rearrange("p t (i c) -> p t i c", c=256)[:, at],
            img_n256.rearrange("p t (v c) -> p t v c", c=256)[:, at, 0::2, :],
            img_n256.rearrange("p t (v c) -> p t v c", c=256)[:, at, 1::2, :],
            op=Alu.subtract,
        )

    # identity for PE transposes
    ident = persist.tile([128, 128], BF16, tag="ident")
    nc.sync.dma_start(ident[:], ident_d[:])

    # weight tiles, [c, a] layout, via PE transposes (PE is idle in the prologue)
    wL128 = persist.tile([128, 4, A_PER], BF16, tag="wL128")  # odd chunk 2i+1
    wD128 = persist.tile([128, 4, A_PER], BF16, tag="wD128")
    wL256 = persist.tile([128, 4, A_PER], BF16, tag="wL256")  # [2i+h]: odd v'=2i+1, half h
    wD256 = persist.tile([128, 4, A_PER], BF16, tag="wD256")
    for at in range(2):
        asl = slice(at * 128, (at + 1) * 128)
        groups = [
            (wL128, [img_n128[:, at, (2 * i + 1) * 128:(2 * i + 2) * 128] for i in range(4)]),
            (wD128, [img_dn128[:, at, i * 128:(i + 1) * 128] for i in range(4)]),
            (wL256, [img_n256[:, at, ((2 * i + 1) * 2 + h) * 128:((2 * i + 1) * 2 + h + 1) * 128]
                     for i in range(2) for h in range(2)]),
            (wD256, [img_dn256[:, at, k * 128:(k + 1) * 128] for k in range(4)]),
        ]
        for gi, (dstT, srcs) in enumerate(groups):
            pt = psum.tile([128, 4, 128], BF16, tag="pL")
            for k, s in enumerate(srcs):
                nc.tensor.transpose(pt[:, k, :], s, ident[:])
            eng = nc.vector if gi % 2 == 0 else nc.scalar
            if gi % 2 == 0:
                nc.vector.tensor_copy(dstT[:, 0:4, asl], pt[:])
            else:
                nc.scalar.copy(dstT[:, 0:4, asl], pt[:])

    # ---------------- cap prep (per quarter) + main loop -----------------------
    # Emission order = scheduler priority: prep(q+1) is emitted before main(q)
    # so the next quarter's normalize/stage/transpose overlaps this quarter's
    # matmul+drain work instead of stalling the PE at the boundary.
    scr_c128 = dram.tile([B, C], BF16, tag="scr_c128")
    scr_c256 = dram.tile([B, C], BF16, tag="scr_c256")

    def prep_quarter(q):
        c128q = persist.tile([128, 8, BQ], BF16, tag=f"capT128_{q}", name=f"capT128_{q}")
        c256q = persist.tile([128, 8, BQ], BF16, tag=f"capT256_{q}", name=f"capT256_{q}")
        for r in range(4):  # row-tiles within quarter
            row0 = q * BQ + r * 128
            x = norm.tile([128, C], F32, tag="cap_in", name=f"cap_in_{q}_{r}")
            nc.sync.dma_start(x[:], cap_d[row0:row0 + 128, :])
            n128 = norm.tile([128, C], BF16, tag="cap_n128", name=f"cap_n128_{q}_{r}")
            n256 = norm.tile([128, C], BF16, tag="cap_n256", name=f"cap_n256_{q}_{r}")
            normalize_tile(x, n128, n256)
            if q == 0:
                # PE transposes straight from SBUF: no DRAM roundtrip latency
                for half, (srcT, dstq) in enumerate(((n128, c128q), (n256, c256q))):
                    for jg in range(2):
                        pt = psum.tile([128, 4, 128], BF16, tag="pL",
                                       name=f"pt_{q}_{r}_{half}_{jg}")
                        for k in range(4):
                            j = jg * 4 + k
                            nc.tensor.transpose(
                                pt[:, k, :], srcT[:, j * 128:(j + 1) * 128], ident[:]
                            )
                        dst = dstq[:, jg * 4:(jg + 1) * 4, r * 128:(r + 1) * 128]
                        if (half + jg) % 2 == 0:
                            nc.vector.tensor_copy(dst, pt[:])
                        else:
                            nc.scalar.copy(dst, pt[:])
            else:
                nc.sync.dma_start(scr_c128[row0:row0 + 128, :], n128[:])
                nc.sync.dma_start(scr_c256[row0:row0 + 128, :], n256[:])
        if q > 0:
            for j in range(8):
                nc.sync.dma_start_transpose(
                    c128q[:, j, :], scr_c128[q * BQ:(q + 1) * BQ, j * 128:(j + 1) * 128]
                )
                nc.sync.dma_start_transpose(
                    c256q[:, j, :], scr_c256[q * BQ:(q + 1) * BQ, j * 128:(j + 1) * 128]
                )
        return c128q, c256q

    def main_quarter(q, c128q, c256q):
        for at in range(2):
            asl = slice(at * 128, (at + 1) * 128)
            m_stage = stage.tile([128, 12, BQ], BF16, tag="m_stage",
                                 name=f"m_stage_{q}_{at}")
            # ---- 128-blocks: t-pair generations, v-pair relu-trick ----
            for tq in range(4):
                mms = []
                for pair in range(4):
                    pL = psum.tile([128, 2, BQ], F32, tag="pL",
                                   name=f"pL_{q}_{at}_{tq}_{pair}")
                    pD = psum.tile([128, 2, BQ], F32, tag="pD",
                                   name=f"pD_{q}_{at}_{tq}_{pair}")
                    for ti in range(2):
                        nc.tensor.matmul(pL[:, ti, :], wL128[:, pair, asl],
                                         c128q[:, 2 * tq + ti, :],
                                         start=True, stop=True)
                    for ti in range(2):
                        nc.tensor.matmul(pD[:, ti, :], wD128[:, pair, asl],
                                         c128q[:, 2 * tq + ti, :],
                                         start=True, stop=True)
                    r = drain.tile([128, 2, BQ], BF16, tag="r",
                                   name=f"r_{q}_{at}_{tq}_{pair}")
                    nc.scalar.activation(r[:], pD[:], Act.Relu)
                    m = drain.tile([128, 2, BQ], BF16, tag="m", bufs=6,
                                   name=f"m_{q}_{at}_{tq}_{pair}")
                    nc.vector.tensor_tensor(m[:], r[:], pL[:], op=Alu.add)
                    mms.append(m)
                mm01 = drain.tile([128, 2, BQ], BF16, tag="mm", bufs=2,
                                  name=f"mm01_{q}_{at}_{tq}")
                nc.vector.tensor_tensor(mm01[:], mms[0][:], mms[1][:], op=Alu.max)
                mm23 = drain.tile([128, 2, BQ], BF16, tag="mm2", bufs=2,
                                  name=f"mm23_{q}_{at}_{tq}")
                nc.vector.tensor_tensor(mm23[:], mms[2][:], mms[3][:], op=Alu.max)
                nc.vector.tensor_tensor(
                    m_stage[:, 2 * tq:2 * tq + 2, :], mm01[:], mm23[:], op=Alu.max
                )
            # ---- 256-blocks: t'-pair generations ----
            for tqp in range(2):
                mis = []
                for i in range(2):
                    pL = psum.tile([128, 2, BQ], F32, tag="pL",
                                   name=f"pL256_{q}_{at}_{tqp}_{i}")
                    pD = psum.tile([128, 2, BQ], F32, tag="pD",
                                   name=f"pD256_{q}_{at}_{tqp}_{i}")
                    for tpi in range(2):
                        tp = 2 * tqp + tpi
                        for h in range(2):
                            nc.tensor.matmul(pL[:, tpi, :], wL256[:, 2 * i + h, asl],
                                             c256q[:, 2 * tp + h, :],
                                             start=(h == 0), stop=(h == 1))
                    for tpi in range(2):
                        tp = 2 * tqp + tpi
                        for h in range(2):
                            nc.tensor.matmul(pD[:, tpi, :], wD256[:, 2 * i + h, asl],
                                             c256q[:, 2 * tp + h, :],
                                             start=(h == 0), stop=(h == 1))
                    r = drain.tile([128, 2, BQ], BF16, tag="r",
                                   name=f"r256_{q}_{at}_{tqp}_{i}")
                    nc.scalar.activation(r[:], pD[:], Act.Relu)
                    m = drain.tile([128, 2, BQ], BF16, tag="m", bufs=6,
                                   name=f"m256_{q}_{at}_{tqp}_{i}")
                    nc.vector.tensor_tensor(m[:], r[:], pL[:], op=Alu.add)
                    mis.append(m)
                nc.vector.tensor_tensor(
                    m_stage[:, 8 + 2 * tqp:8 + 2 * tqp + 2, :],
                    mis[0][:], mis[1][:], op=Alu.max,
                )
            # ---- t-sum via contiguous slab add-tree and writeback ----
            # 12 -> 6 (bf16) -> 3 (fp32) -> 1
            s1 = drain.tile([128, 6, BQ], BF16, tag="s1", bufs=2,
                            name=f"s1_{q}_{at}")
            nc.vector.tensor_tensor(
                s1[:], m_stage[:, 0:6, :], m_stage[:, 6:12, :], op=Alu.add
            )
            s2 = drain.tile([128, 3, BQ], F32, tag="s2", bufs=2,
                            name=f"s2_{q}_{at}")
            nc.vector.tensor_tensor(
                s2[:], s1[:, 0:3, :], s1[:, 3:6, :], op=Alu.add
            )
            acc = drain.tile([128, BQ], F32, tag="acc", bufs=2,
                             name=f"acc_{q}_{at}")
            nc.vector.tensor_tensor(
                acc[:], s2[:, 0, :], s2[:, 1, :], op=Alu.add
            )
            nc.vector.tensor_tensor(
                acc[:], acc[:], s2[:, 2, :], op=Alu.add
            )
            nc.sync.dma_start(out_d[asl, q * BQ:(q + 1) * BQ], acc[:])

    caps = {0: prep_quarter(0)}
    for q in range(NQ):
        if q + 1 < NQ:
            caps[q + 1] = prep_quarter(q + 1)
        main_quarter(q, *caps[q])
        del caps[q]


_NC_CACHE = None




# ---------------------------------------------------------------------------
# Workaround: this container's walrus build rejects instructions with more
# than one sync-wait condition ("Too many sync wait commands").  Split the
# extra waits onto sequencer-only RegisterMove carrier instructions in a BIR
# post-pass, and monkeypatch the compile entry points to apply it.
import json as _json


def _split_multiwaits(bir_bytes: bytes) -> bytes:
    m = _json.loads(bir_bytes)
    uid = [0]

    def carrier(engine, wait, debug):
        uid[0] += 1
        return {
            "debug": debug,
            "engine": engine,
            "ins": [{"dtype": "int32", "kind": "imm_value", "value": 0}],
            "outs": [{"dtype": "int32", "kind": "register_access",
                      "regref": f"{engine}_zero"}],
            "name": f"I-wsplit-{uid[0]}",
            "opcode": "RegisterMove",
            "sync_info": {"on_update": [], "on_wait": [wait]},
        }

    for f in m["functions"]:
        for bb in f["blocks"]:
            out = []
            for inst in bb["instructions"]:
                si = inst.get("sync_info")
                waits = (si or {}).get("on_wait") or []
                eng = inst.get("engine")
                if len(waits) > 1 and eng and eng != "Unassigned":
                    for w in waits[:-1]:
                        out.append(carrier(eng, w, inst.get("debug", 0)))
                    si["on_wait"] = [waits[-1]]
                out.append(inst)
            bb["instructions"] = out
    return _json.dumps(m).encode()


def _install_birpatch():
    import concourse.bass_utils as bu
    import concourse.bass2jax as b2j

    if getattr(bu.compile_bir_kernel, "_wsplit_wrapped", False):
        return
    orig = bu.compile_bir_kernel

    def wrapped(bir_json: bytes, tmpdir: str, neff_name="file.neff"):
        return orig(_split_multiwaits(bir_json), tmpdir, neff_name=neff_name)

    wrapped._wsplit_wrapped = True
    bu.compile_bir_kernel = wrapped
    b2j.compile_bir_kernel = wrapped


def kernel(img_emb: np.ndarray, cap_emb: np.ndarray) -> np.ndarray:
    _install_birpatch()
    from concourse.bass_utils import run_bass_kernel_spmd

    global _NC_CACHE
    if _NC_CACHE is None:
        _NC_CACHE = _build_kernel()
    nc = _NC_CACHE

    import ml_dtypes

    img = np.ascontiguousarray(np.asarray(img_emb, dtype=np.float32))
    cap = np.ascontiguousarray(np.asarray(cap_emb, dtype=np.float32))
    ident = np.eye(128, dtype=ml_dtypes.bfloat16)
    in_maps = [
        {"img": img[k * A_PER:(k + 1) * A_PER], "cap": cap, "ident": ident}
        for k in range(N_CORES)
    ]
    res = run_bass_kernel_spmd(nc, in_maps, core_ids=list(range(N_CORES)))
    return np.concatenate([r["sims"] for r in res.results], axis=0)


if __name__ == "__main__":
    rng = np.random.default_rng(0)
    img = rng.normal(size=(A, C)).astype(np.float32)
    cap = rng.normal(size=(B, C)).astype(np.float32)
    out = kernel(img, cap)
    print("out", out.shape, out.dtype, float(out.min()), float(out.max()))
